# revision 10
# baseline (speedup 1.0000x reference)
"""ChannelAttentionPropagation1D kernel for 8x TRN2 NeuronCores.

Reference computation (per batch b):
  kv[c,d]   = sum_{t,n} key_mem[b,t,n,c] * val_mem[b,t,n,d]    # (64, 64)
  kv_soft   = softmax(kv, axis=c)
  out[n,d]  = alpha * (key_cur[b] @ kv_soft)[n,d] + val_cur[b,n,d]

Sharding (pair-per-batch, 8 cores):
  Core i handles batch i//2. Core 2p contracts the first half of batch
  p's T*n = 131072 memory tokens, core 2p+1 the second half; the two
  16 KB partial kv's are exchanged with a 2-core AllGather (mesh path)
  -- the four pair-exchanges run concurrently, unlike an 8-core
  AllGather chain which serializes on the CC core. Each core then
  computes the output for its own 8192-token slice of batch p.

Precision: key/val memories and val_cur are cast to bf16 on the host,
key_cur (alpha folded) and the softmax weights to fp8-e4m3; kv
accumulates in f32 PSUM and softmax runs in f32. Validated on the
fixed inputs: rel fro err 2.3e-3 vs f64, ~9x under the 2e-2 gate.
Halving the dominant HBM stream is worth ~55us; fp8 halves the
phase-2 PE stream.

Layout notes:
  - phase 1 accumulates kvT[d,c] (PSUM) so the softmax axis c lands on
    the free axis; a PE transpose afterwards yields kv_soft[c,d].
  - phase 2 loads key_cur^T 128-token tiles as PE WEIGHTS (fp8 +
    NumWeights=128 triggers the automatic 4x Fast-Weight-Load, 32
    cycles, fully hidden) and streams the 64 kv_soft columns per tile:
    4096 streamed columns total vs 8192 the other way around. Output
    psum is [128 tok, 64 d] grouped 4 tiles per bank so the vc adds
    run as 16 wide DVE ops; the store layout reshapes directly to
    [tok, d] on the host (token n = 64*p + j). Tiles 0:32 contract on
    PE rows 0:64, tiles 32:64 on rows 64:128 (row tiling).
  - k chunks ride the sync HWDGE ring, v chunks the scalar ring (two
    rings double the outstanding descriptors per SDMA engine); the
    phase-2 inputs key_curT/val_curT queue at the very end of each
    ring so they never delay the contraction, streaming during the
    pair-exchange window instead. ar_in goes out on the otherwise-idle
    gpsimd SWDGE ring so it never queues; readback/mirror/stores ride
    the scalar ring, empty by then.
  - the last 8192 phase-1 tokens are split into 4096/2048/2048 chunks
    so the final chunk's matmul tail exposes <1us after the last HBM
    byte.
"""

import numpy as np
import ml_dtypes

import concourse.bacc as bacc
import concourse.mybir as mybir
import concourse.tile as tile
from concourse import bass_utils, masks

F32 = mybir.dt.float32
BF16 = mybir.dt.bfloat16
FP8 = mybir.dt.float8e4
NPBF16 = np.dtype(ml_dtypes.bfloat16)
NPFP8 = mybir.dt.np(FP8)

N_CORES = 8
N, T, NTOK, C, C2 = 4, 8, 16384, 64, 64
TOT = T * NTOK // 2          # 65536 phase-1 tokens per core
NSL = NTOK // 2              # 8192 phase-2 tokens per core
HNSL = NSL // 2              # 4096 tokens per PE row-group
PAIRS = [[0, 1], [2, 3], [4, 5], [6, 7]]
CHUNKS = [8192] * 7 + [4096, 2048, 2048]
assert sum(CHUNKS) == TOT

_CACHE = {}

# Extra kwargs forwarded to run_bass_kernel_spmd (used by the profiling
# harness to request an NTFF trace; empty for normal correctness runs).
_RUN_OPTS = {}


def _build_program():
    nc = bacc.Bacc(
        "TRN2",
        target_bir_lowering=False,
        debug=False,
        enable_asserts=False,
        num_devices=N_CORES,
    )

    km = nc.dram_tensor("key_mem", [TOT, C], BF16, kind="ExternalInput").ap()
    vm = nc.dram_tensor("val_mem", [TOT, C2], BF16, kind="ExternalInput").ap()
    # key_cur^T (alpha folded), row-tiled: rows 0:64 = channels x tokens
    # 0:4096, rows 64:128 = channels x tokens 4096:8192.
    kct = nc.dram_tensor("key_curT", [128, HNSL], FP8, kind="ExternalInput").ap()
    vct = nc.dram_tensor("val_curT", [128, HNSL], BF16, kind="ExternalInput").ap()
    # output, transposed: [d, tok] row-tiled the same way; host transposes.
    out = nc.dram_tensor("out", [128, HNSL], F32, kind="ExternalOutput").ap()

    with tile.TileContext(nc) as tc:
        with (
            tc.tile_pool(name="persist", bufs=1) as persist,
            tc.tile_pool(name="big", bufs=6) as big,
            tc.tile_pool(name="tmp", bufs=2) as tmp,
            tc.tile_pool(name="ps", bufs=2, space="PSUM") as ps,
            tc.tile_pool(name="dram", bufs=1, space="DRAM") as dram,
        ):
            ident = persist.tile([128, 128], F32)
            masks.make_identity(nc, ident[:])

            kct_sb = persist.tile([128, HNSL], FP8)
            vct_sb = persist.tile([128, HNSL], BF16)
            stg = persist.tile([128, HNSL], F32)
            kvt_sb = persist.tile([C2, C], F32)
            kvt_all = persist.tile([C2, 2 * C], F32)
            kvt_red = persist.tile([C2, C], F32)
            kv_soft = persist.tile([128, C2], FP8)

            # ---- phase 1: partial kvT[d, c], col-tiled 2x ----
            # Even token-tiles accumulate on PE column group 0 (psum rows
            # 0:64), odd tiles on column group 2 (psum rows 64:128).
            kv_ps = ps.tile([128, C], F32, tag="kv", bufs=1)
            n_tiles = TOT // 128
            g = 0  # global 128-token tile index
            t0 = 0
            for ci, ch in enumerate(CHUNKS):
                cols = ch // 128 * C
                k_sb = big.tile([128, 4096], BF16, tag="k")
                v_sb = big.tile([128, 4096], BF16, tag="v")
                nc.sync.dma_start(
                    k_sb[:, 0:cols],
                    km[t0:t0 + ch, :].rearrange("(p a) c -> p (a c)", p=128),
                )
                nc.scalar.dma_start(
                    v_sb[:, 0:cols],
                    vm[t0:t0 + ch, :].rearrange("(p a) c -> p (a c)", p=128),
                )
                t0 += ch
                for a in range(ch // 128):
                    half = a % 2
                    nc.tensor.matmul(
                        kv_ps[64 * half:64 * half + C2, :],
                        lhsT=v_sb[:, a * C2:(a + 1) * C2],
                        rhs=k_sb[:, a * C:(a + 1) * C],
                        start=(g < 2),
                        stop=(g >= n_tiles - 2),
                        tile_position=(0, 64 * half),
                    )
                    g += 1

            # phase-2 inputs queue BEHIND the phase-1 chunks on each ring;
            # they stream during the pair-exchange window.
            nc.sync.dma_start(kct_sb[:], kct)
            nc.scalar.dma_start(vct_sb[:], vct)

            # partial kvT = even-half + odd-half (DVE reads one PSUM
            # operand per instruction: copy then add)
            nc.vector.tensor_copy(kvt_sb[:], kv_ps[0:C2, :])
            nc.vector.tensor_add(kvt_sb[:], kvt_sb[:], kv_ps[64:64 + C2, :])

            # pair exchange: 2-core AllGather (mesh path); Local outputs
            # (Shared is unsupported for <=4-core groups). ar_in rides the
            # idle gpsimd SWDGE ring so it never queues behind kct/vct.
            ar_in = dram.tile([C2, C], F32, tag="ar_in", name="ar_in")
            ar_out = dram.tile([2, C2, C], F32, tag="ar_out", name="ar_out")
            nc.gpsimd.dma_start(ar_in[:], kvt_sb[:])
            nc.gpsimd.collective_compute(
                "AllGather",
                mybir.AluOpType.bypass,
                replica_groups=PAIRS,
                ins=[ar_in.opt()],
                outs=[ar_out.opt()],
            )
            nc.scalar.dma_start(
                kvt_all[:].rearrange("d (r c) -> d r c", r=2),
                ar_out.rearrange("r d c -> d r c"),
            )
            nc.vector.tensor_add(
                kvt_red[:], kvt_all[:, 0:C], kvt_all[:, C:2 * C]
            )

            # softmax over c (free axis)
            neg_mx = tmp.tile([C2, 1], F32)
            nc.vector.reduce_max(
                out=neg_mx[:],
                in_=kvt_red[:],
                axis=mybir.AxisListType.X,
                negate=True,
            )
            ex = tmp.tile([C2, C], F32)
            sm = tmp.tile([C2, 1], F32)
            nc.scalar.activation(
                ex[:],
                kvt_red[:],
                mybir.ActivationFunctionType.Exp,
                bias=neg_mx[:], scale=1.0,
                accum_out=sm[:],
            )
            rv = tmp.tile([C2, 1], F32)
            nc.vector.reciprocal(rv[:], sm[:])
            nc.vector.tensor_scalar_mul(ex[:], ex[:], rv[:])

            # transpose softmaxed kvT to kv[c, d] (transpose-mode matmul
            # writes PSUM partition 0), cast to fp8, and mirror into
            # partitions 64:128 for the second PE quadrant.
            tp = ps.tile([C, C2], F32, tag="tp")
            nc.tensor.transpose(tp[:], ex[:], ident[0:C2, 0:C2])
            nc.vector.tensor_copy(kv_soft[0:C, :], tp[:])
            nc.scalar.dma_start(kv_soft[64:64 + C, :], kv_soft[0:C, :])

            # ---- phase 2: out[tok, d] = key_cur @ kv_soft ----
            # 64 token-tiles of 128; tile j's kct columns sit at
            # j*128:(j+1)*128 of its row-half (tiles 0:32 on kct/kv rows
            # 0:64, tiles 32:64 on rows 64:128). 4 tiles share one psum
            # bank so the vc add is one wide DVE op per group.
            GRP = 4
            for grp in range(32 // GRP):
                for rh in range(2):  # row-half: 0 -> tiles 0:32, 1 -> 32:64
                    rows = slice(64 * rh, 64 * rh + C)
                    o = ps.tile(
                        [128, GRP * C2], F32, tag="o", name=f"o{grp}_{rh}", bufs=3
                    )
                    for t in range(GRP):
                        j = grp * GRP + t
                        nc.tensor.matmul(
                            o[:, t * C2:(t + 1) * C2],
                            lhsT=kct_sb[rows, j * 128:(j + 1) * 128],
                            rhs=kv_soft[rows, :],
                            start=True, stop=True,
                            tile_position=(64 * rh, 0),
                        )
                    col = slice(
                        (32 * rh + grp * GRP) * C2,
                        (32 * rh + grp * GRP + GRP) * C2,
                    )
                    nc.vector.tensor_add(stg[:, col], o[:], vct_sb[:, col])
                    nc.scalar.dma_start(out[:, col], stg[:, col])

    nc.compile()
    return nc


def _get_program():
    if "nc" not in _CACHE:
        _CACHE["nc"] = _build_program()
    return _CACHE["nc"]


def kernel(key_mem, val_mem, key_cur, val_cur, alpha):
    key_mem = np.asarray(key_mem, dtype=np.float32)
    val_mem = np.asarray(val_mem, dtype=np.float32)
    key_cur = np.asarray(key_cur, dtype=np.float32)
    val_cur = np.asarray(val_cur, dtype=np.float32)
    alpha_f = float(np.asarray(alpha).reshape(-1)[0])

    nc = _get_program()

    kc_scaled = (alpha_f * key_cur).astype(np.float32)
    in_maps = []
    for i in range(N_CORES):
        B, H = i // 2, i % 2
        sl = slice(H * NSL, (H + 1) * NSL)
        # kct col j*128+p holds token p*64+j (phase-2 tile j = tokens
        # congruent to j mod 64); rows 0:64 = tiles 0:32, 64:128 = 32:64.
        kct_i = (
            kc_scaled[B, sl].T
            .reshape(C, 128, 64).transpose(0, 2, 1).reshape(C, NSL)
            .reshape(C, 2, HNSL).transpose(1, 0, 2).reshape(128, HNSL)
        )
        # vct/stg/out layout: [p, (j c)] = val/out token 64*p + j.
        vct_i = val_cur[B, sl].reshape(128, HNSL)
        in_maps.append(
            {
                "key_mem": np.ascontiguousarray(
                    key_mem[B, 4 * H:4 * H + 4].reshape(TOT, C)
                ).astype(NPBF16),
                "val_mem": np.ascontiguousarray(
                    val_mem[B, 4 * H:4 * H + 4].reshape(TOT, C2)
                ).astype(NPBF16),
                "key_curT": np.ascontiguousarray(kct_i).astype(NPFP8),
                "val_curT": np.ascontiguousarray(vct_i).astype(NPBF16),
            }
        )

    res = bass_utils.run_bass_kernel_spmd(
        nc, in_maps, core_ids=list(range(N_CORES)), **_RUN_OPTS
    )
    _CACHE["last_result"] = res
    out = np.empty((N, NTOK, C2), dtype=np.float32)
    for i in range(N_CORES):
        B, H = i // 2, i % 2
        o = res.results[i]["out"]  # [128, 4096] f32, [p, (j c)]
        out[B, H * NSL:(H + 1) * NSL] = o.reshape(NSL, C2)
    return out


# revision 11
# speedup vs baseline: 1.0956x; 1.0956x over previous
"""ChannelAttentionPropagation1D kernel for 8x TRN2 NeuronCores.

Reference computation (per batch b):
  kv[c,d]   = sum_{t,n} key_mem[b,t,n,c] * val_mem[b,t,n,d]    # (64, 64)
  kv_soft   = softmax(kv, axis=c)
  out[n,d]  = alpha * (key_cur[b] @ kv_soft)[n,d] + val_cur[b,n,d]

Sharding (pair-per-batch, 8 cores):
  Core i handles batch i//2. Core 2p contracts the first half of batch
  p's T*n = 131072 memory tokens, core 2p+1 the second half; the two
  16 KB partial kv's are exchanged with a 2-core AllGather (mesh path)
  -- the four pair-exchanges run concurrently, unlike an 8-core
  AllGather chain which serializes on the CC core. Each core then
  computes the output for its own 8192-token slice of batch p.

Precision: key/val memories and val_cur are cast to bf16 on the host,
key_cur (alpha folded) and the softmax weights to fp8-e4m3; kv
accumulates in f32 PSUM and softmax runs in f32. Validated on the
fixed inputs: rel fro err 2.3e-3 vs f64, ~9x under the 2e-2 gate.
Halving the dominant HBM stream is worth ~55us; fp8 halves the
phase-2 PE stream.

Layout notes:
  - phase 1 accumulates kvT[d,c] (PSUM) so the softmax axis c lands on
    the free axis; a PE transpose afterwards yields kv_soft[c,d].
  - phase 2 loads key_cur^T 128-token tiles as PE WEIGHTS (fp8 +
    NumWeights=128 triggers the automatic 4x Fast-Weight-Load, 32
    cycles, fully hidden) and streams the 64 kv_soft columns per tile:
    4096 streamed columns total vs 8192 the other way around. Output
    psum is [128 tok, 64 d] grouped 4 tiles per bank so the vc adds
    run as 16 wide DVE ops; the store layout reshapes directly to
    [tok, d] on the host (token n = 64*p + j). Tiles 0:32 contract on
    PE rows 0:64, tiles 32:64 on rows 64:128 (row tiling).
  - k chunks ride the sync HWDGE ring, v chunks the scalar ring (two
    rings double the outstanding descriptors per SDMA engine); the
    phase-2 inputs key_curT/val_curT queue at the very end of each
    ring so they never delay the contraction, streaming during the
    pair-exchange window instead. ar_in goes out on the otherwise-idle
    gpsimd SWDGE ring so it never queues; readback/mirror/stores ride
    the scalar ring, empty by then.
  - the last 8192 phase-1 tokens are split into 4096/2048/2048 chunks
    so the final chunk's matmul tail exposes <1us after the last HBM
    byte.
"""

import numpy as np
import ml_dtypes

import concourse.bacc as bacc
import concourse.mybir as mybir
import concourse.tile as tile
from concourse import bass_utils, masks

F32 = mybir.dt.float32
BF16 = mybir.dt.bfloat16
FP8 = mybir.dt.float8e4
NPBF16 = np.dtype(ml_dtypes.bfloat16)
NPFP8 = mybir.dt.np(FP8)

N_CORES = 8
N, T, NTOK, C, C2 = 4, 8, 16384, 64, 64
TOT = T * NTOK // 2          # 65536 phase-1 tokens per core
NSL = NTOK // 2              # 8192 phase-2 tokens per core
HNSL = NSL // 2              # 4096 tokens per PE row-group
PAIRS = [[0, 1], [2, 3], [4, 5], [6, 7]]
CHUNKS = [8192] * 7 + [4096, 2048, 2048]
assert sum(CHUNKS) == TOT

_CACHE = {}

# Extra kwargs forwarded to run_bass_kernel_spmd (used by the profiling
# harness to request an NTFF trace; empty for normal correctness runs).
_RUN_OPTS = {}


def _build_program():
    nc = bacc.Bacc(
        "TRN2",
        target_bir_lowering=False,
        debug=False,
        enable_asserts=False,
        num_devices=N_CORES,
    )

    km = nc.dram_tensor("key_mem", [TOT, C], BF16, kind="ExternalInput").ap()
    vm = nc.dram_tensor("val_mem", [TOT, C2], BF16, kind="ExternalInput").ap()
    # key_cur^T (alpha folded), row-tiled: rows 0:64 = channels x tokens
    # 0:4096, rows 64:128 = channels x tokens 4096:8192.
    kct = nc.dram_tensor("key_curT", [128, HNSL], FP8, kind="ExternalInput").ap()
    vct = nc.dram_tensor("val_curT", [128, HNSL], BF16, kind="ExternalInput").ap()
    # output, transposed: [d, tok] row-tiled the same way; host transposes.
    out = nc.dram_tensor("out", [128, HNSL], F32, kind="ExternalOutput").ap()

    with tile.TileContext(nc) as tc:
        with (
            tc.tile_pool(name="persist", bufs=1) as persist,
            tc.tile_pool(name="big", bufs=6) as big,
            tc.tile_pool(name="tmp", bufs=2) as tmp,
            tc.tile_pool(name="ps", bufs=2, space="PSUM") as ps,
            tc.tile_pool(name="dram", bufs=1, space="DRAM") as dram,
        ):
            ident = persist.tile([128, 128], F32)
            masks.make_identity(nc, ident[:])

            kct_sb = persist.tile([128, HNSL], FP8)
            vct_sb = persist.tile([128, HNSL], BF16)
            stg = persist.tile([128, HNSL], F32)
            kvt_sb = persist.tile([C2, C], F32)
            kvt_all = persist.tile([C2, 2 * C], F32)
            kvt_red = persist.tile([C2, C], F32)
            kv_soft = persist.tile([128, C2], FP8)

            # ---- phase 1: partial kvT[d, c], col-tiled 2x ----
            # Even token-tiles accumulate on PE column group 0 (psum rows
            # 0:64), odd tiles on column group 2 (psum rows 64:128).
            kv_ps = ps.tile([128, C], F32, tag="kv", bufs=1)
            n_tiles = TOT // 128
            g = 0  # global 128-token tile index
            t0 = 0
            for ci, ch in enumerate(CHUNKS):
                cols = ch // 128 * C
                k_sb = big.tile([128, 4096], BF16, tag="k")
                v_sb = big.tile([128, 4096], BF16, tag="v")
                nc.sync.dma_start(
                    k_sb[:, 0:cols],
                    km[t0:t0 + ch, :].rearrange("(p a) c -> p (a c)", p=128),
                )
                nc.scalar.dma_start(
                    v_sb[:, 0:cols],
                    vm[t0:t0 + ch, :].rearrange("(p a) c -> p (a c)", p=128),
                )
                t0 += ch
                for a in range(ch // 128):
                    half = a % 2
                    nc.tensor.matmul(
                        kv_ps[64 * half:64 * half + C2, :],
                        lhsT=v_sb[:, a * C2:(a + 1) * C2],
                        rhs=k_sb[:, a * C:(a + 1) * C],
                        start=(g < 2),
                        stop=(g >= n_tiles - 2),
                        tile_position=(0, 64 * half),
                    )
                    g += 1

            # phase-2 inputs queue BEHIND the phase-1 chunks on each ring;
            # they stream during the pair-exchange window.
            nc.sync.dma_start(kct_sb[:], kct)
            nc.scalar.dma_start(vct_sb[:], vct)

            # partial kvT = even-half + odd-half (DVE reads one PSUM
            # operand per instruction: copy then add)
            nc.vector.tensor_copy(kvt_sb[:], kv_ps[0:C2, :])
            nc.vector.tensor_add(kvt_sb[:], kvt_sb[:], kv_ps[64:64 + C2, :])

            # pair exchange: 2-core AllGather (mesh path); Local outputs
            # (Shared is unsupported for <=4-core groups). ar_in rides the
            # idle gpsimd SWDGE ring so it never queues behind kct/vct.
            ar_in = dram.tile([C2, C], F32, tag="ar_in", name="ar_in")
            ar_out = dram.tile([2, C2, C], F32, tag="ar_out", name="ar_out")
            nc.gpsimd.dma_start(ar_in[:], kvt_sb[:])
            nc.gpsimd.collective_compute(
                "AllGather",
                mybir.AluOpType.bypass,
                replica_groups=PAIRS,
                ins=[ar_in.opt()],
                outs=[ar_out.opt()],
            )
            nc.scalar.dma_start(
                kvt_all[:].rearrange("d (r c) -> d r c", r=2),
                ar_out.rearrange("r d c -> d r c"),
            )
            nc.vector.tensor_add(
                kvt_red[:], kvt_all[:, 0:C], kvt_all[:, C:2 * C]
            )

            # softmax over c (free axis)
            neg_mx = tmp.tile([C2, 1], F32)
            nc.vector.reduce_max(
                out=neg_mx[:],
                in_=kvt_red[:],
                axis=mybir.AxisListType.X,
                negate=True,
            )
            ex = tmp.tile([C2, C], F32)
            sm = tmp.tile([C2, 1], F32)
            nc.scalar.activation(
                ex[:],
                kvt_red[:],
                mybir.ActivationFunctionType.Exp,
                bias=neg_mx[:], scale=1.0,
                accum_out=sm[:],
            )
            rv = tmp.tile([C2, 1], F32)
            nc.vector.reciprocal(rv[:], sm[:])
            nc.vector.tensor_scalar_mul(ex[:], ex[:], rv[:])

            # transpose softmaxed kvT to kv[c, d] (transpose-mode matmul
            # writes PSUM partition 0), cast to fp8, and mirror into
            # partitions 64:128 for the second PE quadrant.
            tp = ps.tile([C, C2], F32, tag="tp")
            nc.tensor.transpose(tp[:], ex[:], ident[0:C2, 0:C2])
            nc.vector.tensor_copy(kv_soft[0:C, :], tp[:])
            nc.scalar.dma_start(kv_soft[64:64 + C, :], kv_soft[0:C, :])

            # ---- phase 2: out[tok, d] = key_cur @ kv_soft ----
            # 64 token-tiles of 128; tile j's kct columns sit at
            # j*128:(j+1)*128 of its row-half (tiles 0:32 on kct/kv rows
            # 0:64, tiles 32:64 on rows 64:128). 4 tiles share one psum
            # bank so the vc add is one wide DVE op per group.
            GRP = 4
            for grp in range(32 // GRP):
                for rh in range(2):  # row-half: 0 -> tiles 0:32, 1 -> 32:64
                    rows = slice(64 * rh, 64 * rh + C)
                    o = ps.tile(
                        [128, GRP * C2], F32, tag="o", name=f"o{grp}_{rh}", bufs=3
                    )
                    for t in range(GRP):
                        j = grp * GRP + t
                        nc.tensor.matmul(
                            o[:, t * C2:(t + 1) * C2],
                            lhsT=kct_sb[rows, j * 128:(j + 1) * 128],
                            rhs=kv_soft[rows, :],
                            start=True, stop=True,
                            tile_position=(64 * rh, 0),
                        )
                    col = slice(
                        (32 * rh + grp * GRP) * C2,
                        (32 * rh + grp * GRP + GRP) * C2,
                    )
                    nc.vector.tensor_add(stg[:, col], o[:], vct_sb[:, col])
                    # stores batch 2 groups (512 KB) and alternate between
                    # the sync and scalar rings -- both idle by now -- so
                    # the 2.1 MB output drains at full rate.
                    if grp % 2 == 1:
                        scol = slice(
                            (32 * rh + (grp - 1) * GRP) * C2,
                            (32 * rh + (grp + 1) * GRP) * C2,
                        )
                        eng = nc.sync if rh == 0 else nc.scalar
                        eng.dma_start(out[:, scol], stg[:, scol])

    nc.compile()
    return nc


def _get_program():
    if "nc" not in _CACHE:
        _CACHE["nc"] = _build_program()
    return _CACHE["nc"]


def kernel(key_mem, val_mem, key_cur, val_cur, alpha):
    key_mem = np.asarray(key_mem, dtype=np.float32)
    val_mem = np.asarray(val_mem, dtype=np.float32)
    key_cur = np.asarray(key_cur, dtype=np.float32)
    val_cur = np.asarray(val_cur, dtype=np.float32)
    alpha_f = float(np.asarray(alpha).reshape(-1)[0])

    nc = _get_program()

    kc_scaled = (alpha_f * key_cur).astype(np.float32)
    in_maps = []
    for i in range(N_CORES):
        B, H = i // 2, i % 2
        sl = slice(H * NSL, (H + 1) * NSL)
        # kct col j*128+p holds token p*64+j (phase-2 tile j = tokens
        # congruent to j mod 64); rows 0:64 = tiles 0:32, 64:128 = 32:64.
        kct_i = (
            kc_scaled[B, sl].T
            .reshape(C, 128, 64).transpose(0, 2, 1).reshape(C, NSL)
            .reshape(C, 2, HNSL).transpose(1, 0, 2).reshape(128, HNSL)
        )
        # vct/stg/out layout: [p, (j c)] = val/out token 64*p + j.
        vct_i = val_cur[B, sl].reshape(128, HNSL)
        in_maps.append(
            {
                "key_mem": np.ascontiguousarray(
                    key_mem[B, 4 * H:4 * H + 4].reshape(TOT, C)
                ).astype(NPBF16),
                "val_mem": np.ascontiguousarray(
                    val_mem[B, 4 * H:4 * H + 4].reshape(TOT, C2)
                ).astype(NPBF16),
                "key_curT": np.ascontiguousarray(kct_i).astype(NPFP8),
                "val_curT": np.ascontiguousarray(vct_i).astype(NPBF16),
            }
        )

    res = bass_utils.run_bass_kernel_spmd(
        nc, in_maps, core_ids=list(range(N_CORES)), **_RUN_OPTS
    )
    _CACHE["last_result"] = res
    out = np.empty((N, NTOK, C2), dtype=np.float32)
    for i in range(N_CORES):
        B, H = i // 2, i % 2
        o = res.results[i]["out"]  # [128, 4096] f32, [p, (j c)]
        out[B, H * NSL:(H + 1) * NSL] = o.reshape(NSL, C2)
    return out


# revision 15
# speedup vs baseline: 1.1110x; 1.0140x over previous
"""ChannelAttentionPropagation1D kernel for 8x TRN2 NeuronCores.

Reference computation (per batch b):
  kv[c,d]   = sum_{t,n} key_mem[b,t,n,c] * val_mem[b,t,n,d]    # (64, 64)
  kv_soft   = softmax(kv, axis=c)
  out[n,d]  = alpha * (key_cur[b] @ kv_soft)[n,d] + val_cur[b,n,d]

Sharding (pair-per-batch, 8 cores):
  Core i handles batch i//2. Core 2p contracts the first half of batch
  p's T*n = 131072 memory tokens, core 2p+1 the second half; the two
  16 KB partial kv's are exchanged with a 2-core AllGather (mesh path)
  -- the four pair-exchanges run concurrently, unlike an 8-core
  AllGather chain which serializes on the CC core. Each core then
  computes the output for its own 8192-token slice of batch p.

Precision: key/val memories and val_cur are cast to bf16 on the host,
key_cur (alpha folded) and the softmax weights to fp8-e4m3; kv
accumulates in f32 PSUM and softmax runs in f32. Validated on the
fixed inputs: rel fro err 2.3e-3 vs f64, ~9x under the 2e-2 gate.
Halving the dominant HBM stream is worth ~55us; fp8 halves the
phase-2 PE stream.

Layout notes:
  - phase 1 accumulates kvT[d,c] (PSUM) so the softmax axis c lands on
    the free axis; a PE transpose afterwards yields kv_soft[c,d].
  - phase 2 loads key_cur^T 128-token tiles as PE WEIGHTS (fp8 +
    NumWeights=128 triggers the automatic 4x Fast-Weight-Load, 32
    cycles, fully hidden) and streams the 64 kv_soft columns per tile:
    4096 streamed columns total vs 8192 the other way around. Output
    psum is [128 tok, 64 d] grouped 4 tiles per bank so the vc adds
    run as 16 wide DVE ops; the store layout reshapes directly to
    [tok, d] on the host (token n = 64*p + j). Tiles 0:32 contract on
    PE rows 0:64, tiles 32:64 on rows 64:128 (row tiling).
  - k chunks ride the sync HWDGE ring, v chunks the scalar ring (two
    rings double the outstanding descriptors per SDMA engine); the
    phase-2 inputs key_curT/val_curT queue at the very end of each
    ring so they never delay the contraction, streaming during the
    pair-exchange window instead. ar_in goes out on the otherwise-idle
    gpsimd SWDGE ring so it never queues; readback/mirror/stores ride
    the scalar ring, empty by then.
  - the last 8192 phase-1 tokens are split into 4096/2048/2048 chunks
    so the final chunk's matmul tail exposes <1us after the last HBM
    byte.
"""

import numpy as np
import ml_dtypes

import concourse.bacc as bacc
import concourse.mybir as mybir
import concourse.tile as tile
from concourse import bass_utils, masks

F32 = mybir.dt.float32
BF16 = mybir.dt.bfloat16
FP8 = mybir.dt.float8e4
NPBF16 = np.dtype(ml_dtypes.bfloat16)
NPFP8 = mybir.dt.np(FP8)

N_CORES = 8
N, T, NTOK, C, C2 = 4, 8, 16384, 64, 64
TOT = T * NTOK // 2          # 65536 phase-1 tokens per core
NSL = NTOK // 2              # 8192 phase-2 tokens per core
HNSL = NSL // 2              # 4096 tokens per PE row-group
PAIRS = [[0, 1], [2, 3], [4, 5], [6, 7]]
CHUNKS = [8192] * 7 + [4096, 2048, 2048]
assert sum(CHUNKS) == TOT

_CACHE = {}

# Extra kwargs forwarded to run_bass_kernel_spmd (used by the profiling
# harness to request an NTFF trace; empty for normal correctness runs).
_RUN_OPTS = {}


def _build_program():
    nc = bacc.Bacc(
        "TRN2",
        target_bir_lowering=False,
        debug=False,
        enable_asserts=False,
        num_devices=N_CORES,
    )

    km = nc.dram_tensor("key_mem", [TOT, C], BF16, kind="ExternalInput").ap()
    vm = nc.dram_tensor("val_mem", [TOT, C2], BF16, kind="ExternalInput").ap()
    # key_cur^T (alpha folded), row-tiled: rows 0:64 = channels x tokens
    # 0:4096, rows 64:128 = channels x tokens 4096:8192.
    kct = nc.dram_tensor("key_curT", [128, HNSL], FP8, kind="ExternalInput").ap()
    vct = nc.dram_tensor("val_curT", [128, HNSL], BF16, kind="ExternalInput").ap()
    # output, transposed: [d, tok] row-tiled the same way; host transposes.
    out = nc.dram_tensor("out", [128, HNSL], F32, kind="ExternalOutput").ap()

    with tile.TileContext(nc) as tc:
        with (
            tc.tile_pool(name="persist", bufs=1) as persist,
            tc.tile_pool(name="big", bufs=6) as big,
            tc.tile_pool(name="tmp", bufs=2) as tmp,
            tc.tile_pool(name="ps", bufs=2, space="PSUM") as ps,
            tc.tile_pool(name="dram", bufs=1, space="DRAM") as dram,
        ):
            ident = persist.tile([128, 128], F32)
            masks.make_identity(nc, ident[:])

            kct_sb = persist.tile([128, HNSL], FP8)
            vct_sb = persist.tile([128, HNSL], BF16)
            stg = persist.tile([128, HNSL], F32)
            kvt_sb = persist.tile([C2, C], F32)
            kvt_all = persist.tile([C2, 2 * C], F32)
            kvt_red = persist.tile([C2, C], F32)
            kv_soft = persist.tile([128, C2], FP8)

            # ---- phase 1: partial kvT[d, c], col-tiled 2x ----
            # Even token-tiles accumulate on PE column group 0 (psum rows
            # 0:64), odd tiles on column group 2 (psum rows 64:128).
            kv_ps = ps.tile([128, C], F32, tag="kv", bufs=1)
            n_tiles = TOT // 128
            g = 0  # global 128-token tile index
            t0 = 0
            for ci, ch in enumerate(CHUNKS):
                cols = ch // 128 * C
                k_sb = big.tile([128, 4096], BF16, tag="k")
                v_sb = big.tile([128, 4096], BF16, tag="v")
                nc.sync.dma_start(
                    k_sb[:, 0:cols],
                    km[t0:t0 + ch, :].rearrange("(p a) c -> p (a c)", p=128),
                )
                nc.scalar.dma_start(
                    v_sb[:, 0:cols],
                    vm[t0:t0 + ch, :].rearrange("(p a) c -> p (a c)", p=128),
                )
                t0 += ch
                for a in range(ch // 128):
                    half = a % 2
                    nc.tensor.matmul(
                        kv_ps[64 * half:64 * half + C2, :],
                        lhsT=v_sb[:, a * C2:(a + 1) * C2],
                        rhs=k_sb[:, a * C:(a + 1) * C],
                        start=(g < 2),
                        stop=(g >= n_tiles - 2),
                        tile_position=(0, 64 * half),
                    )
                    g += 1

            # phase-2 inputs queue BEHIND the phase-1 chunks on each ring;
            # they stream during the pair-exchange window.
            nc.sync.dma_start(kct_sb[:], kct)
            nc.scalar.dma_start(vct_sb[:], vct)

            # partial kvT = even-half + odd-half (DVE reads one PSUM
            # operand per instruction: copy then add)
            nc.vector.tensor_copy(kvt_sb[:], kv_ps[0:C2, :])
            nc.vector.tensor_add(kvt_sb[:], kvt_sb[:], kv_ps[64:64 + C2, :])

            # pair exchange: 2-core AllGather (mesh path); Local outputs
            # (Shared is unsupported for <=4-core groups). ar_in rides the
            # idle gpsimd SWDGE ring so it never queues behind kct/vct.
            ar_in = dram.tile([C2, C], F32, tag="ar_in", name="ar_in")
            ar_out = dram.tile([2, C2, C], F32, tag="ar_out", name="ar_out")
            nc.gpsimd.dma_start(ar_in[:], kvt_sb[:])
            nc.gpsimd.collective_compute(
                "AllGather",
                mybir.AluOpType.bypass,
                replica_groups=PAIRS,
                ins=[ar_in.opt()],
                outs=[ar_out.opt()],
            )
            nc.scalar.dma_start(
                kvt_all[:].rearrange("d (r c) -> d r c", r=2),
                ar_out.rearrange("r d c -> d r c"),
            )
            nc.vector.tensor_add(
                kvt_red[:], kvt_all[:, 0:C], kvt_all[:, C:2 * C]
            )

            # softmax over c (free axis)
            neg_mx = tmp.tile([C2, 1], F32)
            nc.vector.reduce_max(
                out=neg_mx[:],
                in_=kvt_red[:],
                axis=mybir.AxisListType.X,
                negate=True,
            )
            ex = tmp.tile([C2, C], F32)
            sm = tmp.tile([C2, 1], F32)
            nc.scalar.activation(
                ex[:],
                kvt_red[:],
                mybir.ActivationFunctionType.Exp,
                bias=neg_mx[:], scale=1.0,
                accum_out=sm[:],
            )
            rv = tmp.tile([C2, 1], F32)
            nc.vector.reciprocal(rv[:], sm[:])
            nc.vector.tensor_scalar_mul(ex[:], ex[:], rv[:])

            # transpose softmaxed kvT to kv[c, d] (transpose-mode matmul
            # writes PSUM partition 0), cast to fp8, and mirror into
            # partitions 64:128 for the second PE quadrant.
            tp = ps.tile([C, C2], F32, tag="tp")
            nc.tensor.transpose(tp[:], ex[:], ident[0:C2, 0:C2])
            # mirror via two DVE copies (partition-offset writes) -- an
            # SBUF->SBUF DMA here costs ~2us of completion latency that
            # stalls the row-half-1 matmuls.
            nc.vector.tensor_copy(kv_soft[0:C, :], tp[:])
            nc.vector.tensor_copy(kv_soft[64:64 + C, :], tp[:])

            # ---- phase 2: out[tok, d] = key_cur @ kv_soft ----
            # 64 token-tiles of 128; tile j's kct columns sit at
            # j*128:(j+1)*128 of its row-half (tiles 0:32 on kct/kv rows
            # 0:64, tiles 32:64 on rows 64:128). 4 tiles share one psum
            # bank so the vc add is one wide DVE op per group.
            GRP = 8
            for grp in range(32 // GRP):
                for rh in range(2):  # row-half: 0 -> tiles 0:32, 1 -> 32:64
                    rows = slice(64 * rh, 64 * rh + C)
                    o = ps.tile(
                        [128, GRP * C2], F32, tag="o", name=f"o{grp}_{rh}", bufs=3
                    )
                    for t in range(GRP):
                        j = grp * GRP + t
                        nc.tensor.matmul(
                            o[:, t * C2:(t + 1) * C2],
                            lhsT=kct_sb[rows, j * 128:(j + 1) * 128],
                            rhs=kv_soft[rows, :],
                            start=True, stop=True,
                            tile_position=(64 * rh, 0),
                        )
                    col = slice(
                        (32 * rh + grp * GRP) * C2,
                        (32 * rh + grp * GRP + GRP) * C2,
                    )
                    nc.vector.tensor_add(stg[:, col], o[:], vct_sb[:, col])
                    # stores batch 2 groups (512 KB) and alternate between
                    # the sync and scalar rings -- both idle by now -- so
                    # the 2.1 MB output drains at full rate.
                    if grp % 2 == 1:
                        scol = slice(
                            (32 * rh + (grp - 1) * GRP) * C2,
                            (32 * rh + (grp + 1) * GRP) * C2,
                        )
                        eng = nc.sync if rh == 0 else nc.scalar
                        eng.dma_start(out[:, scol], stg[:, scol])

    nc.compile()
    return nc


def _get_program():
    if "nc" not in _CACHE:
        _CACHE["nc"] = _build_program()
    return _CACHE["nc"]


def kernel(key_mem, val_mem, key_cur, val_cur, alpha):
    key_mem = np.asarray(key_mem, dtype=np.float32)
    val_mem = np.asarray(val_mem, dtype=np.float32)
    key_cur = np.asarray(key_cur, dtype=np.float32)
    val_cur = np.asarray(val_cur, dtype=np.float32)
    alpha_f = float(np.asarray(alpha).reshape(-1)[0])

    nc = _get_program()

    kc_scaled = (alpha_f * key_cur).astype(np.float32)
    in_maps = []
    for i in range(N_CORES):
        B, H = i // 2, i % 2
        sl = slice(H * NSL, (H + 1) * NSL)
        # kct col j*128+p holds token p*64+j (phase-2 tile j = tokens
        # congruent to j mod 64); rows 0:64 = tiles 0:32, 64:128 = 32:64.
        kct_i = (
            kc_scaled[B, sl].T
            .reshape(C, 128, 64).transpose(0, 2, 1).reshape(C, NSL)
            .reshape(C, 2, HNSL).transpose(1, 0, 2).reshape(128, HNSL)
        )
        # vct/stg/out layout: [p, (j c)] = val/out token 64*p + j.
        vct_i = val_cur[B, sl].reshape(128, HNSL)
        in_maps.append(
            {
                "key_mem": np.ascontiguousarray(
                    key_mem[B, 4 * H:4 * H + 4].reshape(TOT, C)
                ).astype(NPBF16),
                "val_mem": np.ascontiguousarray(
                    val_mem[B, 4 * H:4 * H + 4].reshape(TOT, C2)
                ).astype(NPBF16),
                "key_curT": np.ascontiguousarray(kct_i).astype(NPFP8),
                "val_curT": np.ascontiguousarray(vct_i).astype(NPBF16),
            }
        )

    res = bass_utils.run_bass_kernel_spmd(
        nc, in_maps, core_ids=list(range(N_CORES)), **_RUN_OPTS
    )
    _CACHE["last_result"] = res
    out = np.empty((N, NTOK, C2), dtype=np.float32)
    for i in range(N_CORES):
        B, H = i // 2, i % 2
        o = res.results[i]["out"]  # [128, 4096] f32, [p, (j c)]
        out[B, H * NSL:(H + 1) * NSL] = o.reshape(NSL, C2)
    return out


# revision 20
# speedup vs baseline: 1.1274x; 1.0148x over previous
"""ChannelAttentionPropagation1D kernel for 8x TRN2 NeuronCores.

Reference computation (per batch b):
  kv[c,d]   = sum_{t,n} key_mem[b,t,n,c] * val_mem[b,t,n,d]    # (64, 64)
  kv_soft   = softmax(kv, axis=c)
  out[n,d]  = alpha * (key_cur[b] @ kv_soft)[n,d] + val_cur[b,n,d]

Sharding (pair-per-batch, 8 cores):
  Core i handles batch i//2. Core 2p contracts the first half of batch
  p's T*n = 131072 memory tokens, core 2p+1 the second half; the two
  16 KB partial kv's are exchanged with a 2-core AllGather (mesh path)
  -- the four pair-exchanges run concurrently, unlike an 8-core
  AllGather chain which serializes on the CC core. Each core then
  computes the output for its own 8192-token slice of batch p.

Precision: key/val memories and val_cur are cast to bf16 on the host,
key_cur (alpha folded) and the softmax weights to fp8-e4m3; kv
accumulates in f32 PSUM and softmax runs in f32. Validated on the
fixed inputs: rel fro err 2.3e-3 vs f64, ~9x under the 2e-2 gate.
Halving the dominant HBM stream is worth ~55us; fp8 halves the
phase-2 PE stream.

Layout notes:
  - phase 1 accumulates kvT[d,c] (PSUM) so the softmax axis c lands on
    the free axis; a PE transpose afterwards yields kv_soft[c,d].
  - phase 2 loads key_cur^T 128-token tiles as PE WEIGHTS (fp8 +
    NumWeights=128 triggers the automatic 4x Fast-Weight-Load, 32
    cycles, fully hidden) and streams the 64 kv_soft columns per tile:
    4096 streamed columns total vs 8192 the other way around. Output
    psum is [128 tok, 64 d] grouped 4 tiles per bank so the vc adds
    run as 16 wide DVE ops; the store layout reshapes directly to
    [tok, d] on the host (token n = 64*p + j). Tiles 0:32 contract on
    PE rows 0:64, tiles 32:64 on rows 64:128 (row tiling).
  - k chunks ride the sync HWDGE ring, v chunks the scalar ring (two
    rings double the outstanding descriptors per SDMA engine); the
    phase-2 inputs key_curT/val_curT queue at the very end of each
    ring so they never delay the contraction, streaming during the
    pair-exchange window instead. ar_in goes out on the otherwise-idle
    gpsimd SWDGE ring so it never queues; readback/mirror/stores ride
    the scalar ring, empty by then.
  - the last 8192 phase-1 tokens are split into 4096/2048/2048 chunks
    so the final chunk's matmul tail exposes <1us after the last HBM
    byte.
"""

import numpy as np
import ml_dtypes

import concourse.bacc as bacc
import concourse.mybir as mybir
import concourse.tile as tile
from concourse import bass_utils, masks

F32 = mybir.dt.float32
BF16 = mybir.dt.bfloat16
FP8 = mybir.dt.float8e4
NPBF16 = np.dtype(ml_dtypes.bfloat16)
NPFP8 = mybir.dt.np(FP8)

N_CORES = 8
N, T, NTOK, C, C2 = 4, 8, 16384, 64, 64
TOT = T * NTOK // 2          # 65536 phase-1 tokens per core
NSL = NTOK // 2              # 8192 phase-2 tokens per core
HNSL = NSL // 2              # 4096 tokens per PE row-group
PAIRS = [[0, 1], [2, 3], [4, 5], [6, 7]]
CHUNKS = [16384] * 3 + [8192, 4096, 2048, 2048]
assert sum(CHUNKS) == TOT

_CACHE = {}

# Extra kwargs forwarded to run_bass_kernel_spmd (used by the profiling
# harness to request an NTFF trace; empty for normal correctness runs).
_RUN_OPTS = {}


def _build_program():
    nc = bacc.Bacc(
        "TRN2",
        target_bir_lowering=False,
        debug=False,
        enable_asserts=False,
        num_devices=N_CORES,
    )

    km = nc.dram_tensor("key_mem", [TOT, C], BF16, kind="ExternalInput").ap()
    vm = nc.dram_tensor("val_mem", [TOT, C2], BF16, kind="ExternalInput").ap()
    # key_cur^T (alpha folded), row-tiled: rows 0:64 = channels x tokens
    # 0:4096, rows 64:128 = channels x tokens 4096:8192.
    kct = nc.dram_tensor("key_curT", [128, HNSL], FP8, kind="ExternalInput").ap()
    vct = nc.dram_tensor("val_curT", [128, HNSL], BF16, kind="ExternalInput").ap()
    # output, transposed: [d, tok] row-tiled the same way; host transposes.
    out = nc.dram_tensor("out", [128, HNSL], F32, kind="ExternalOutput").ap()

    with tile.TileContext(nc) as tc:
        with (
            tc.tile_pool(name="persist", bufs=1) as persist,
            tc.tile_pool(name="big", bufs=4) as big,
            tc.tile_pool(name="tmp", bufs=2) as tmp,
            tc.tile_pool(name="ps", bufs=2, space="PSUM") as ps,
            tc.tile_pool(name="dram", bufs=1, space="DRAM") as dram,
        ):
            ident = persist.tile([128, 128], F32)
            masks.make_identity(nc, ident[:])

            kct_sb = persist.tile([128, HNSL], FP8)
            vct_sb = persist.tile([128, HNSL], BF16)
            stg = persist.tile([128, HNSL], F32)
            kvt_sb = persist.tile([C2, C], F32)
            kvt_all = persist.tile([C2, 2 * C], F32)
            kvt_red = persist.tile([C2, C], F32)
            kv_soft = persist.tile([128, C2], FP8)

            # ---- phase 1: partial kvT[d, c], col-tiled 2x ----
            # Even token-tiles accumulate on PE column group 0 (psum rows
            # 0:64), odd tiles on column group 2 (psum rows 64:128).
            kv_ps = ps.tile([128, C], F32, tag="kv", bufs=1)
            n_tiles = TOT // 128
            g = 0  # global 128-token tile index
            t0 = 0
            for ci, ch in enumerate(CHUNKS):
                cols = ch // 128 * C
                k_sb = big.tile([128, 8192], BF16, tag="k")
                v_sb = big.tile([128, 8192], BF16, tag="v")
                nc.sync.dma_start(
                    k_sb[:, 0:cols],
                    km[t0:t0 + ch, :].rearrange("(p a) c -> p (a c)", p=128),
                )
                nc.scalar.dma_start(
                    v_sb[:, 0:cols],
                    vm[t0:t0 + ch, :].rearrange("(p a) c -> p (a c)", p=128),
                )
                t0 += ch
                for a in range(ch // 128):
                    half = a % 2
                    nc.tensor.matmul(
                        kv_ps[64 * half:64 * half + C2, :],
                        lhsT=v_sb[:, a * C2:(a + 1) * C2],
                        rhs=k_sb[:, a * C:(a + 1) * C],
                        start=(g < 2),
                        stop=(g >= n_tiles - 2),
                        tile_position=(0, 64 * half),
                    )
                    g += 1

            # phase-2 inputs queue BEHIND the phase-1 chunks on each ring;
            # they stream during the pair-exchange window.
            nc.sync.dma_start(kct_sb[:], kct)
            nc.scalar.dma_start(vct_sb[:], vct)

            # partial kvT = even-half + odd-half (DVE reads one PSUM
            # operand per instruction: copy then add)
            nc.vector.tensor_copy(kvt_sb[:], kv_ps[0:C2, :])
            nc.vector.tensor_add(kvt_sb[:], kvt_sb[:], kv_ps[64:64 + C2, :])

            # pair exchange: 2-core AllGather (mesh path); Local outputs
            # (Shared is unsupported for <=4-core groups). ar_in rides the
            # idle gpsimd SWDGE ring so it never queues behind kct/vct.
            ar_in = dram.tile([C2, C], F32, tag="ar_in", name="ar_in")
            ar_out = dram.tile([2, C2, C], F32, tag="ar_out", name="ar_out")
            nc.gpsimd.dma_start(ar_in[:], kvt_sb[:])
            nc.gpsimd.collective_compute(
                "AllGather",
                mybir.AluOpType.bypass,
                replica_groups=PAIRS,
                ins=[ar_in.opt()],
                outs=[ar_out.opt()],
            )
            nc.scalar.dma_start(
                kvt_all[:].rearrange("d (r c) -> d r c", r=2),
                ar_out.rearrange("r d c -> d r c"),
            )
            nc.vector.tensor_add(
                kvt_red[:], kvt_all[:, 0:C], kvt_all[:, C:2 * C]
            )

            # softmax over c (free axis)
            neg_mx = tmp.tile([C2, 1], F32)
            nc.vector.reduce_max(
                out=neg_mx[:],
                in_=kvt_red[:],
                axis=mybir.AxisListType.X,
                negate=True,
            )
            ex = tmp.tile([C2, C], F32)
            sm = tmp.tile([C2, 1], F32)
            nc.scalar.activation(
                ex[:],
                kvt_red[:],
                mybir.ActivationFunctionType.Exp,
                bias=neg_mx[:], scale=1.0,
                accum_out=sm[:],
            )
            rv = tmp.tile([C2, 1], F32)
            nc.vector.reciprocal(rv[:], sm[:])
            nc.vector.tensor_scalar_mul(ex[:], ex[:], rv[:])

            # transpose softmaxed kvT to kv[c, d] (transpose-mode matmul
            # writes PSUM partition 0), cast to fp8, and mirror into
            # partitions 64:128 for the second PE quadrant.
            tp = ps.tile([C, C2], F32, tag="tp")
            nc.tensor.transpose(tp[:], ex[:], ident[0:C2, 0:C2])
            # mirror via two DVE copies (partition-offset writes) -- an
            # SBUF->SBUF DMA here costs ~2us of completion latency that
            # stalls the row-half-1 matmuls.
            nc.vector.tensor_copy(kv_soft[0:C, :], tp[:])
            nc.vector.tensor_copy(kv_soft[64:64 + C, :], tp[:])

            # ---- phase 2: out[tok, d] = key_cur @ kv_soft ----
            # 64 token-tiles of 128; tile j's kct columns sit at
            # j*128:(j+1)*128 of its row-half (tiles 0:32 on kct/kv rows
            # 0:64, tiles 32:64 on rows 64:128). 4 tiles share one psum
            # bank so the vc add is one wide DVE op per group.
            GRP = 8
            for grp in range(32 // GRP):
                for rh in range(2):  # row-half: 0 -> tiles 0:32, 1 -> 32:64
                    rows = slice(64 * rh, 64 * rh + C)
                    o = ps.tile(
                        [128, GRP * C2], F32, tag="o", name=f"o{grp}_{rh}", bufs=3
                    )
                    for t in range(GRP):
                        j = grp * GRP + t
                        nc.tensor.matmul(
                            o[:, t * C2:(t + 1) * C2],
                            lhsT=kct_sb[rows, j * 128:(j + 1) * 128],
                            rhs=kv_soft[rows, :],
                            start=True, stop=True,
                            tile_position=(64 * rh, 0),
                        )
                    col = slice(
                        (32 * rh + grp * GRP) * C2,
                        (32 * rh + grp * GRP + GRP) * C2,
                    )
                    nc.vector.tensor_add(stg[:, col], o[:], vct_sb[:, col])
                    # stores batch 2 groups (512 KB) and alternate between
                    # the sync and scalar rings -- both idle by now -- so
                    # the 2.1 MB output drains at full rate.
                    if grp % 2 == 1:
                        scol = slice(
                            (32 * rh + (grp - 1) * GRP) * C2,
                            (32 * rh + (grp + 1) * GRP) * C2,
                        )
                        eng = nc.sync if rh == 0 else nc.scalar
                        eng.dma_start(out[:, scol], stg[:, scol])

    nc.compile()
    return nc


def _get_program():
    if "nc" not in _CACHE:
        _CACHE["nc"] = _build_program()
    return _CACHE["nc"]


def kernel(key_mem, val_mem, key_cur, val_cur, alpha):
    key_mem = np.asarray(key_mem, dtype=np.float32)
    val_mem = np.asarray(val_mem, dtype=np.float32)
    key_cur = np.asarray(key_cur, dtype=np.float32)
    val_cur = np.asarray(val_cur, dtype=np.float32)
    alpha_f = float(np.asarray(alpha).reshape(-1)[0])

    nc = _get_program()

    kc_scaled = (alpha_f * key_cur).astype(np.float32)
    in_maps = []
    for i in range(N_CORES):
        B, H = i // 2, i % 2
        sl = slice(H * NSL, (H + 1) * NSL)
        # kct col j*128+p holds token p*64+j (phase-2 tile j = tokens
        # congruent to j mod 64); rows 0:64 = tiles 0:32, 64:128 = 32:64.
        kct_i = (
            kc_scaled[B, sl].T
            .reshape(C, 128, 64).transpose(0, 2, 1).reshape(C, NSL)
            .reshape(C, 2, HNSL).transpose(1, 0, 2).reshape(128, HNSL)
        )
        # vct/stg/out layout: [p, (j c)] = val/out token 64*p + j.
        vct_i = val_cur[B, sl].reshape(128, HNSL)
        in_maps.append(
            {
                "key_mem": np.ascontiguousarray(
                    key_mem[B, 4 * H:4 * H + 4].reshape(TOT, C)
                ).astype(NPBF16),
                "val_mem": np.ascontiguousarray(
                    val_mem[B, 4 * H:4 * H + 4].reshape(TOT, C2)
                ).astype(NPBF16),
                "key_curT": np.ascontiguousarray(kct_i).astype(NPFP8),
                "val_curT": np.ascontiguousarray(vct_i).astype(NPBF16),
            }
        )

    res = bass_utils.run_bass_kernel_spmd(
        nc, in_maps, core_ids=list(range(N_CORES)), **_RUN_OPTS
    )
    _CACHE["last_result"] = res
    out = np.empty((N, NTOK, C2), dtype=np.float32)
    for i in range(N_CORES):
        B, H = i // 2, i % 2
        o = res.results[i]["out"]  # [128, 4096] f32, [p, (j c)]
        out[B, H * NSL:(H + 1) * NSL] = o.reshape(NSL, C2)
    return out


# revision 24
# speedup vs baseline: 1.1902x; 1.0558x over previous
"""ChannelAttentionPropagation1D kernel for 8x TRN2 NeuronCores.

Reference computation (per batch b):
  kv[c,d]   = sum_{t,n} key_mem[b,t,n,c] * val_mem[b,t,n,d]    # (64, 64)
  kv_soft   = softmax(kv, axis=c)
  out[n,d]  = alpha * (key_cur[b] @ kv_soft)[n,d] + val_cur[b,n,d]

Sharding (pair-per-batch, 8 cores):
  Core i handles batch i//2. Core 2p contracts the first half of batch
  p's T*n = 131072 memory tokens, core 2p+1 the second half; the two
  16 KB partial kv's are exchanged with a 2-core AllGather (mesh path)
  -- the four pair-exchanges run concurrently, unlike an 8-core
  AllGather chain which serializes on the CC core. Each core then
  computes the output for its own 8192-token slice of batch p.

Precision: key/val memories and val_cur are cast to bf16 on the host,
key_cur (alpha folded) and the softmax weights to fp8-e4m3; kv
accumulates in f32 PSUM and softmax runs in f32. Validated on the
fixed inputs: rel fro err 2.3e-3 vs f64, ~9x under the 2e-2 gate.
Halving the dominant HBM stream is worth ~55us; fp8 halves the
phase-2 PE stream.

Layout notes:
  - phase 1 accumulates kvT[d,c] (PSUM) so the softmax axis c lands on
    the free axis; a PE transpose afterwards yields kv_soft[c,d].
  - phase 2 loads key_cur^T 128-token tiles as PE WEIGHTS (fp8 +
    NumWeights=128 triggers the automatic 4x Fast-Weight-Load, 32
    cycles, fully hidden) and streams the 64 kv_soft columns per tile:
    4096 streamed columns total vs 8192 the other way around. Output
    psum is [128 tok, 64 d] grouped 4 tiles per bank so the vc adds
    run as 16 wide DVE ops; the store layout reshapes directly to
    [tok, d] on the host (token n = 64*p + j). Tiles 0:32 contract on
    PE rows 0:64, tiles 32:64 on rows 64:128 (row tiling).
  - k chunks ride the sync HWDGE ring, v chunks the scalar ring (two
    rings double the outstanding descriptors per SDMA engine); the
    phase-2 inputs key_curT/val_curT queue at the very end of each
    ring so they never delay the contraction, streaming during the
    pair-exchange window instead. ar_in goes out on the otherwise-idle
    gpsimd SWDGE ring so it never queues; readback/mirror/stores ride
    the scalar ring, empty by then.
  - the last 8192 phase-1 tokens are split into 4096/2048/2048 chunks
    so the final chunk's matmul tail exposes <1us after the last HBM
    byte.
"""

import numpy as np
import ml_dtypes

import concourse.bacc as bacc
import concourse.mybir as mybir
import concourse.tile as tile
from concourse import bass_utils, masks

F32 = mybir.dt.float32
BF16 = mybir.dt.bfloat16
FP8 = mybir.dt.float8e4
NPBF16 = np.dtype(ml_dtypes.bfloat16)
NPFP8 = mybir.dt.np(FP8)

N_CORES = 8
N, T, NTOK, C, C2 = 4, 8, 16384, 64, 64
TOT = T * NTOK // 2          # 65536 phase-1 tokens per core
NSL = NTOK // 2              # 8192 phase-2 tokens per core
HNSL = NSL // 2              # 4096 tokens per PE row-group
PAIRS = [[0, 1], [2, 3], [4, 5], [6, 7]]
CHUNKS = [16384] * 3 + [8192, 4096, 2048, 2048]
assert sum(CHUNKS) == TOT

_CACHE = {}

# Extra kwargs forwarded to run_bass_kernel_spmd (used by the profiling
# harness to request an NTFF trace; empty for normal correctness runs).
_RUN_OPTS = {}


def _build_program():
    nc = bacc.Bacc(
        "TRN2",
        target_bir_lowering=False,
        debug=False,
        enable_asserts=False,
        num_devices=N_CORES,
    )

    km = nc.dram_tensor("key_mem", [TOT, C], BF16, kind="ExternalInput").ap()
    vm = nc.dram_tensor("val_mem", [TOT, C2], BF16, kind="ExternalInput").ap()
    # key_cur^T (alpha folded), row-tiled: rows 0:64 = channels x tokens
    # 0:4096, rows 64:128 = channels x tokens 4096:8192.
    kct = nc.dram_tensor("key_curT", [128, HNSL], FP8, kind="ExternalInput").ap()
    vct = nc.dram_tensor("val_curT", [128, HNSL], BF16, kind="ExternalInput").ap()
    # output, transposed: [d, tok] row-tiled the same way; host transposes.
    out = nc.dram_tensor("out", [128, HNSL], F32, kind="ExternalOutput").ap()

    with tile.TileContext(nc) as tc:
        with (
            tc.tile_pool(name="persist", bufs=1) as persist,
            tc.tile_pool(name="big", bufs=4) as big,
            tc.tile_pool(name="tmp", bufs=2) as tmp,
            tc.tile_pool(name="ps", bufs=2, space="PSUM") as ps,
            tc.tile_pool(name="dram", bufs=1, space="DRAM") as dram,
        ):
            ident = persist.tile([128, 128], F32)
            masks.make_identity(nc, ident[:])

            kct_sb = persist.tile([128, HNSL], FP8)
            vct_sb = persist.tile([128, HNSL], BF16)
            stg = persist.tile([128, HNSL], F32)
            kvt_sb = persist.tile([C2, C], F32)
            kvt_all = persist.tile([C2, 2 * C], F32)
            kvt_red = persist.tile([C2, C], F32)
            kv_soft = persist.tile([128, C2], FP8)

            # ---- phase 1: partial kvT[d, c], col-tiled 2x ----
            # Even token-tiles accumulate on PE column group 0 (psum rows
            # 0:64), odd tiles on column group 2 (psum rows 64:128).
            kv_ps = ps.tile([128, C], F32, tag="kv", bufs=1)
            n_tiles = TOT // 128
            g = 0  # global 128-token tile index
            t0 = 0
            for ci, ch in enumerate(CHUNKS):
                cols = ch // 128 * C
                k_sb = big.tile([128, 8192], BF16, tag="k")
                v_sb = big.tile([128, 8192], BF16, tag="v")
                nc.sync.dma_start(
                    k_sb[:, 0:cols],
                    km[t0:t0 + ch, :].rearrange("(p a) c -> p (a c)", p=128),
                )
                nc.scalar.dma_start(
                    v_sb[:, 0:cols],
                    vm[t0:t0 + ch, :].rearrange("(p a) c -> p (a c)", p=128),
                )
                t0 += ch
                for a in range(ch // 128):
                    half = a % 2
                    nc.tensor.matmul(
                        kv_ps[64 * half:64 * half + C2, :],
                        lhsT=v_sb[:, a * C2:(a + 1) * C2],
                        rhs=k_sb[:, a * C:(a + 1) * C],
                        start=(g < 2),
                        stop=(g >= n_tiles - 2),
                        tile_position=(0, 64 * half),
                    )
                    g += 1

            # phase-2 inputs queue BEHIND the phase-1 chunks on each ring;
            # they stream during the pair-exchange window.
            nc.sync.dma_start(kct_sb[:], kct)
            nc.scalar.dma_start(vct_sb[:], vct)

            # partial kvT = even-half + odd-half (DVE reads one PSUM
            # operand per instruction: copy then add)
            nc.vector.tensor_copy(kvt_sb[:], kv_ps[0:C2, :])
            nc.vector.tensor_add(kvt_sb[:], kvt_sb[:], kv_ps[64:64 + C2, :])

            # pair exchange: 2-core AllGather (mesh path); Local outputs
            # (Shared is unsupported for <=4-core groups). ar_in rides the
            # idle gpsimd SWDGE ring so it never queues behind kct/vct.
            ar_in = dram.tile([C2, C], F32, tag="ar_in", name="ar_in")
            ar_out = dram.tile([2, C2, C], F32, tag="ar_out", name="ar_out")
            nc.sync.dma_start(ar_in[:], kvt_sb[:])
            nc.gpsimd.collective_compute(
                "AllGather",
                mybir.AluOpType.bypass,
                replica_groups=PAIRS,
                ins=[ar_in.opt()],
                outs=[ar_out.opt()],
            )
            nc.scalar.dma_start(
                kvt_all[:].rearrange("d (r c) -> d r c", r=2),
                ar_out.rearrange("r d c -> d r c"),
            )
            nc.vector.tensor_add(
                kvt_red[:], kvt_all[:, 0:C], kvt_all[:, C:2 * C]
            )

            # softmax over c (free axis)
            neg_mx = tmp.tile([C2, 1], F32)
            nc.vector.reduce_max(
                out=neg_mx[:],
                in_=kvt_red[:],
                axis=mybir.AxisListType.X,
                negate=True,
            )
            ex = tmp.tile([C2, C], F32)
            sm = tmp.tile([C2, 1], F32)
            nc.scalar.activation(
                ex[:],
                kvt_red[:],
                mybir.ActivationFunctionType.Exp,
                bias=neg_mx[:], scale=1.0,
                accum_out=sm[:],
            )
            rv = tmp.tile([C2, 1], F32)
            nc.vector.reciprocal(rv[:], sm[:])
            nc.vector.tensor_scalar_mul(ex[:], ex[:], rv[:])

            # transpose softmaxed kvT to kv[c, d] (transpose-mode matmul
            # writes PSUM partition 0), cast to fp8, and mirror into
            # partitions 64:128 for the second PE quadrant.
            tp = ps.tile([C, C2], F32, tag="tp")
            nc.tensor.transpose(tp[:], ex[:], ident[0:C2, 0:C2])
            # mirror via two DVE copies (partition-offset writes) -- an
            # SBUF->SBUF DMA here costs ~2us of completion latency that
            # stalls the row-half-1 matmuls.
            nc.vector.tensor_copy(kv_soft[0:C, :], tp[:])
            nc.vector.tensor_copy(kv_soft[64:64 + C, :], tp[:])

            # ---- phase 2: out[tok, d] = key_cur @ kv_soft ----
            # 64 token-tiles of 128; tile j's kct columns sit at
            # j*128:(j+1)*128 of its row-half (tiles 0:32 on kct/kv rows
            # 0:64, tiles 32:64 on rows 64:128). 4 tiles share one psum
            # bank so the vc add is one wide DVE op per group.
            GRP = 8
            for grp in range(32 // GRP):
                for rh in range(2):  # row-half: 0 -> tiles 0:32, 1 -> 32:64
                    rows = slice(64 * rh, 64 * rh + C)
                    o = ps.tile(
                        [128, GRP * C2], F32, tag="o", name=f"o{grp}_{rh}", bufs=3
                    )
                    for t in range(GRP):
                        j = grp * GRP + t
                        nc.tensor.matmul(
                            o[:, t * C2:(t + 1) * C2],
                            lhsT=kct_sb[rows, j * 128:(j + 1) * 128],
                            rhs=kv_soft[rows, :],
                            start=True, stop=True,
                            tile_position=(64 * rh, 0),
                        )
                    col = slice(
                        (32 * rh + grp * GRP) * C2,
                        (32 * rh + grp * GRP + GRP) * C2,
                    )
                    nc.vector.tensor_add(stg[:, col], o[:], vct_sb[:, col])
                    # stores alternate between the sync and scalar rings
                    # (both idle by now) so the 2.1 MB output drains at
                    # full rate and the final piece is only 256 KB.
                    eng = nc.sync if rh == 0 else nc.scalar
                    eng.dma_start(out[:, col], stg[:, col])

    nc.compile()
    return nc


def _get_program():
    if "nc" not in _CACHE:
        _CACHE["nc"] = _build_program()
    return _CACHE["nc"]


def kernel(key_mem, val_mem, key_cur, val_cur, alpha):
    key_mem = np.asarray(key_mem, dtype=np.float32)
    val_mem = np.asarray(val_mem, dtype=np.float32)
    key_cur = np.asarray(key_cur, dtype=np.float32)
    val_cur = np.asarray(val_cur, dtype=np.float32)
    alpha_f = float(np.asarray(alpha).reshape(-1)[0])

    nc = _get_program()

    kc_scaled = (alpha_f * key_cur).astype(np.float32)
    in_maps = []
    for i in range(N_CORES):
        B, H = i // 2, i % 2
        sl = slice(H * NSL, (H + 1) * NSL)
        # kct col j*128+p holds token p*64+j (phase-2 tile j = tokens
        # congruent to j mod 64); rows 0:64 = tiles 0:32, 64:128 = 32:64.
        kct_i = (
            kc_scaled[B, sl].T
            .reshape(C, 128, 64).transpose(0, 2, 1).reshape(C, NSL)
            .reshape(C, 2, HNSL).transpose(1, 0, 2).reshape(128, HNSL)
        )
        # vct/stg/out layout: [p, (j c)] = val/out token 64*p + j.
        vct_i = val_cur[B, sl].reshape(128, HNSL)
        in_maps.append(
            {
                "key_mem": np.ascontiguousarray(
                    key_mem[B, 4 * H:4 * H + 4].reshape(TOT, C)
                ).astype(NPBF16),
                "val_mem": np.ascontiguousarray(
                    val_mem[B, 4 * H:4 * H + 4].reshape(TOT, C2)
                ).astype(NPBF16),
                "key_curT": np.ascontiguousarray(kct_i).astype(NPFP8),
                "val_curT": np.ascontiguousarray(vct_i).astype(NPBF16),
            }
        )

    res = bass_utils.run_bass_kernel_spmd(
        nc, in_maps, core_ids=list(range(N_CORES)), **_RUN_OPTS
    )
    _CACHE["last_result"] = res
    out = np.empty((N, NTOK, C2), dtype=np.float32)
    for i in range(N_CORES):
        B, H = i // 2, i % 2
        o = res.results[i]["out"]  # [128, 4096] f32, [p, (j c)]
        out[B, H * NSL:(H + 1) * NSL] = o.reshape(NSL, C2)
    return out


# revision 26
# speedup vs baseline: 1.2156x; 1.0213x over previous
"""ChannelAttentionPropagation1D kernel for 8x TRN2 NeuronCores.

Reference computation (per batch b):
  kv[c,d]   = sum_{t,n} key_mem[b,t,n,c] * val_mem[b,t,n,d]    # (64, 64)
  kv_soft   = softmax(kv, axis=c)
  out[n,d]  = alpha * (key_cur[b] @ kv_soft)[n,d] + val_cur[b,n,d]

Sharding (pair-per-batch, 8 cores):
  Core i handles batch i//2. Core 2p contracts the first half of batch
  p's T*n = 131072 memory tokens, core 2p+1 the second half; the two
  16 KB partial kv's are exchanged with a 2-core AllGather (mesh path)
  -- the four pair-exchanges run concurrently, unlike an 8-core
  AllGather chain which serializes on the CC core. Each core then
  computes the output for its own 8192-token slice of batch p.

Precision: key/val memories and val_cur are cast to bf16 on the host,
key_cur (alpha folded) and the softmax weights to fp8-e4m3; kv
accumulates in f32 PSUM and softmax runs in f32. Validated on the
fixed inputs: rel fro err 2.3e-3 vs f64, ~9x under the 2e-2 gate.
Halving the dominant HBM stream is worth ~55us; fp8 halves the
phase-2 PE stream.

Layout notes:
  - phase 1 accumulates kvT[d,c] (PSUM) so the softmax axis c lands on
    the free axis; a PE transpose afterwards yields kv_soft[c,d].
  - phase 2 loads key_cur^T 128-token tiles as PE WEIGHTS (fp8 +
    NumWeights=128 triggers the automatic 4x Fast-Weight-Load, 32
    cycles, fully hidden) and streams the 64 kv_soft columns per tile:
    4096 streamed columns total vs 8192 the other way around. Output
    psum is [128 tok, 64 d] grouped 4 tiles per bank so the vc adds
    run as 16 wide DVE ops; the store layout reshapes directly to
    [tok, d] on the host (token n = 64*p + j). Tiles 0:32 contract on
    PE rows 0:64, tiles 32:64 on rows 64:128 (row tiling).
  - k chunks ride the sync HWDGE ring, v chunks the scalar ring (two
    rings double the outstanding descriptors per SDMA engine); the
    phase-2 inputs key_curT/val_curT queue at the very end of each
    ring so they never delay the contraction, streaming during the
    pair-exchange window instead. ar_in goes out on the otherwise-idle
    gpsimd SWDGE ring so it never queues; readback/mirror/stores ride
    the scalar ring, empty by then.
  - the last 8192 phase-1 tokens are split into 4096/2048/2048 chunks
    so the final chunk's matmul tail exposes <1us after the last HBM
    byte.
"""

import numpy as np
import ml_dtypes

import concourse.bacc as bacc
import concourse.mybir as mybir
import concourse.tile as tile
from concourse import bass_utils, masks

F32 = mybir.dt.float32
BF16 = mybir.dt.bfloat16
FP8 = mybir.dt.float8e4
NPBF16 = np.dtype(ml_dtypes.bfloat16)
NPFP8 = mybir.dt.np(FP8)

N_CORES = 8
N, T, NTOK, C, C2 = 4, 8, 16384, 64, 64
TOT = T * NTOK // 2          # 65536 phase-1 tokens per core
NSL = NTOK // 2              # 8192 phase-2 tokens per core
HNSL = NSL // 2              # 4096 tokens per PE row-group
PAIRS = [[0, 1], [2, 3], [4, 5], [6, 7]]
CHUNKS = [16384] * 3 + [8192, 4096, 2048, 2048]
assert sum(CHUNKS) == TOT

_CACHE = {}

# Extra kwargs forwarded to run_bass_kernel_spmd (used by the profiling
# harness to request an NTFF trace; empty for normal correctness runs).
_RUN_OPTS = {}


def _build_program():
    nc = bacc.Bacc(
        "TRN2",
        target_bir_lowering=False,
        debug=False,
        enable_asserts=False,
        num_devices=N_CORES,
    )

    km = nc.dram_tensor("key_mem", [TOT, C], BF16, kind="ExternalInput").ap()
    vm = nc.dram_tensor("val_mem", [TOT, C2], BF16, kind="ExternalInput").ap()
    # key_cur^T (alpha folded), row-tiled: rows 0:64 = channels x tokens
    # 0:4096, rows 64:128 = channels x tokens 4096:8192.
    kct = nc.dram_tensor("key_curT", [128, HNSL], FP8, kind="ExternalInput").ap()
    vct = nc.dram_tensor("val_curT", [128, HNSL], BF16, kind="ExternalInput").ap()
    # output, transposed: [d, tok] row-tiled the same way; host transposes.
    out = nc.dram_tensor("out", [128, HNSL], F32, kind="ExternalOutput").ap()

    with tile.TileContext(nc) as tc:
        with (
            tc.tile_pool(name="persist", bufs=1) as persist,
            tc.tile_pool(name="big", bufs=5) as big,
            tc.tile_pool(name="tmp", bufs=2) as tmp,
            tc.tile_pool(name="ps", bufs=2, space="PSUM") as ps,
            tc.tile_pool(name="dram", bufs=1, space="DRAM") as dram,
        ):
            ident = persist.tile([128, 128], F32)
            masks.make_identity(nc, ident[:])

            kct_sb = persist.tile([128, HNSL], FP8)
            vct_sb = persist.tile([128, HNSL], BF16)
            stg = persist.tile([128, HNSL], F32)
            kvt_sb = persist.tile([C2, C], F32)
            kvt_all = persist.tile([C2, 2 * C], F32)
            kvt_red = persist.tile([C2, C], F32)
            kv_soft = persist.tile([128, C2], FP8)

            # ---- phase 1: partial kvT[d, c], col-tiled 2x ----
            # Even token-tiles accumulate on PE column group 0 (psum rows
            # 0:64), odd tiles on column group 2 (psum rows 64:128).
            kv_ps = ps.tile([128, C], F32, tag="kv", bufs=1)
            n_tiles = TOT // 128
            g = 0  # global 128-token tile index
            t0 = 0
            for ci, ch in enumerate(CHUNKS):
                cols = ch // 128 * C
                k_sb = big.tile([128, 8192], BF16, tag="k")
                v_sb = big.tile([128, 8192], BF16, tag="v")
                nc.sync.dma_start(
                    k_sb[:, 0:cols],
                    km[t0:t0 + ch, :].rearrange("(p a) c -> p (a c)", p=128),
                )
                nc.scalar.dma_start(
                    v_sb[:, 0:cols],
                    vm[t0:t0 + ch, :].rearrange("(p a) c -> p (a c)", p=128),
                )
                t0 += ch
                for a in range(ch // 128):
                    half = a % 2
                    nc.tensor.matmul(
                        kv_ps[64 * half:64 * half + C2, :],
                        lhsT=v_sb[:, a * C2:(a + 1) * C2],
                        rhs=k_sb[:, a * C:(a + 1) * C],
                        start=(g < 2),
                        stop=(g >= n_tiles - 2),
                        tile_position=(0, 64 * half),
                    )
                    g += 1

            # phase-2 inputs queue BEHIND the phase-1 chunks on each ring;
            # they stream during the pair-exchange window.
            nc.sync.dma_start(kct_sb[:], kct)
            nc.scalar.dma_start(vct_sb[:], vct)

            # partial kvT = even-half + odd-half (DVE reads one PSUM
            # operand per instruction: copy then add)
            nc.vector.tensor_copy(kvt_sb[:], kv_ps[0:C2, :])
            nc.vector.tensor_add(kvt_sb[:], kvt_sb[:], kv_ps[64:64 + C2, :])

            # pair exchange: 2-core AllGather (mesh path); Local outputs
            # (Shared is unsupported for <=4-core groups). ar_in rides the
            # idle gpsimd SWDGE ring so it never queues behind kct/vct.
            ar_in = dram.tile([C2, C], F32, tag="ar_in", name="ar_in")
            ar_out = dram.tile([2, C2, C], F32, tag="ar_out", name="ar_out")
            nc.gpsimd.dma_start(ar_in[:], kvt_sb[:])
            nc.gpsimd.collective_compute(
                "AllGather",
                mybir.AluOpType.bypass,
                replica_groups=PAIRS,
                ins=[ar_in.opt()],
                outs=[ar_out.opt()],
            )
            nc.scalar.dma_start(
                kvt_all[:].rearrange("d (r c) -> d r c", r=2),
                ar_out.rearrange("r d c -> d r c"),
            )
            nc.vector.tensor_add(
                kvt_red[:], kvt_all[:, 0:C], kvt_all[:, C:2 * C]
            )

            # softmax over c (free axis)
            neg_mx = tmp.tile([C2, 1], F32)
            nc.vector.reduce_max(
                out=neg_mx[:],
                in_=kvt_red[:],
                axis=mybir.AxisListType.X,
                negate=True,
            )
            ex = tmp.tile([C2, C], F32)
            sm = tmp.tile([C2, 1], F32)
            nc.scalar.activation(
                ex[:],
                kvt_red[:],
                mybir.ActivationFunctionType.Exp,
                bias=neg_mx[:], scale=1.0,
                accum_out=sm[:],
            )
            rv = tmp.tile([C2, 1], F32)
            nc.vector.reciprocal(rv[:], sm[:])
            nc.vector.tensor_scalar_mul(ex[:], ex[:], rv[:])

            # transpose softmaxed kvT to kv[c, d] (transpose-mode matmul
            # writes PSUM partition 0), cast to fp8, and mirror into
            # partitions 64:128 for the second PE quadrant.
            tp = ps.tile([C, C2], F32, tag="tp")
            nc.tensor.transpose(tp[:], ex[:], ident[0:C2, 0:C2])
            # mirror via two DVE copies (partition-offset writes) -- an
            # SBUF->SBUF DMA here costs ~2us of completion latency that
            # stalls the row-half-1 matmuls.
            nc.vector.tensor_copy(kv_soft[0:C, :], tp[:])
            nc.vector.tensor_copy(kv_soft[64:64 + C, :], tp[:])

            # ---- phase 2: out[tok, d] = key_cur @ kv_soft ----
            # 64 token-tiles of 128; tile j's kct columns sit at
            # j*128:(j+1)*128 of its row-half (tiles 0:32 on kct/kv rows
            # 0:64, tiles 32:64 on rows 64:128). 4 tiles share one psum
            # bank so the vc add is one wide DVE op per group.
            GRP = 8
            for grp in range(32 // GRP):
                for rh in range(2):  # row-half: 0 -> tiles 0:32, 1 -> 32:64
                    rows = slice(64 * rh, 64 * rh + C)
                    o = ps.tile(
                        [128, GRP * C2], F32, tag="o", name=f"o{grp}_{rh}", bufs=3
                    )
                    for t in range(GRP):
                        j = grp * GRP + t
                        nc.tensor.matmul(
                            o[:, t * C2:(t + 1) * C2],
                            lhsT=kct_sb[rows, j * 128:(j + 1) * 128],
                            rhs=kv_soft[rows, :],
                            start=True, stop=True,
                            tile_position=(64 * rh, 0),
                        )
                    col = slice(
                        (32 * rh + grp * GRP) * C2,
                        (32 * rh + grp * GRP + GRP) * C2,
                    )
                    nc.vector.tensor_add(stg[:, col], o[:], vct_sb[:, col])
                    # stores alternate between the sync and scalar rings
                    # (both idle by now) so the 2.1 MB output drains at
                    # full rate and the final piece is only 256 KB.
                    eng = nc.sync if rh == 0 else nc.scalar
                    eng.dma_start(out[:, col], stg[:, col])

    nc.compile()
    return nc


def _get_program():
    if "nc" not in _CACHE:
        _CACHE["nc"] = _build_program()
    return _CACHE["nc"]


def kernel(key_mem, val_mem, key_cur, val_cur, alpha):
    key_mem = np.asarray(key_mem, dtype=np.float32)
    val_mem = np.asarray(val_mem, dtype=np.float32)
    key_cur = np.asarray(key_cur, dtype=np.float32)
    val_cur = np.asarray(val_cur, dtype=np.float32)
    alpha_f = float(np.asarray(alpha).reshape(-1)[0])

    nc = _get_program()

    kc_scaled = (alpha_f * key_cur).astype(np.float32)
    in_maps = []
    for i in range(N_CORES):
        B, H = i // 2, i % 2
        sl = slice(H * NSL, (H + 1) * NSL)
        # kct col j*128+p holds token p*64+j (phase-2 tile j = tokens
        # congruent to j mod 64); rows 0:64 = tiles 0:32, 64:128 = 32:64.
        kct_i = (
            kc_scaled[B, sl].T
            .reshape(C, 128, 64).transpose(0, 2, 1).reshape(C, NSL)
            .reshape(C, 2, HNSL).transpose(1, 0, 2).reshape(128, HNSL)
        )
        # vct/stg/out layout: [p, (j c)] = val/out token 64*p + j.
        vct_i = val_cur[B, sl].reshape(128, HNSL)
        in_maps.append(
            {
                "key_mem": np.ascontiguousarray(
                    key_mem[B, 4 * H:4 * H + 4].reshape(TOT, C)
                ).astype(NPBF16),
                "val_mem": np.ascontiguousarray(
                    val_mem[B, 4 * H:4 * H + 4].reshape(TOT, C2)
                ).astype(NPBF16),
                "key_curT": np.ascontiguousarray(kct_i).astype(NPFP8),
                "val_curT": np.ascontiguousarray(vct_i).astype(NPBF16),
            }
        )

    res = bass_utils.run_bass_kernel_spmd(
        nc, in_maps, core_ids=list(range(N_CORES)), **_RUN_OPTS
    )
    _CACHE["last_result"] = res
    out = np.empty((N, NTOK, C2), dtype=np.float32)
    for i in range(N_CORES):
        B, H = i // 2, i % 2
        o = res.results[i]["out"]  # [128, 4096] f32, [p, (j c)]
        out[B, H * NSL:(H + 1) * NSL] = o.reshape(NSL, C2)
    return out
